# revision 1
# baseline (speedup 1.0000x reference)
"""Trainium2 Bass kernel for ContinuousWaveletLayer (CWT energy), v2.

Reference computation:
  bank = Morlet wavelet bank [32 scales, Lmax=256] (static)
  coef[b,s,t] = 'same' conv of x[b,:] (len 8192) with bank[s,:]
  out[b,s]    = mean_t(coef^2) * softmax(scale_weights)[s]

v2 strategy (vs bf16 3-matmul Toeplitz baseline):
  * fp8(e4m3) weights+x with DoubleRow matmuls (K=256 per pass).
  * Scale s has support width 8s centered at 127.5 in the 256 buffer, so
    scales 1..16 fit a single K=256 window IF the x window is shifted by
    +64: one DoubleRow MM per (scale, 4-block group) instead of three
    bf16 MMs.  Large scales (17..32) take a DoubleRow pass + one plain
    fp8 K=128 pass.  Each core gets 2 small + 2 large scales (balanced).
  * The fp8 quantization error in the energies is dominated by a
    deterministic per-scale ||w_q||^2 / ||w||^2 factor (plus a per-row
    ||x_q||^2 factor) which the host divides out exactly.
  * Input DMA is split into a weights chunk + 8 x-slab chunks laid out
    chunk-major so each conv MM's rhs sits inside exactly one chunk
    (the walrus build allows one sync wait per instruction).  A tiny
    PE "guard" matmul per chunk carries the DMA wait; warmup matmuls on
    the weights chunk run during the DMA to trip the HAM clock gate.
"""

import sys
from contextlib import ExitStack

import numpy as np

sys.path.insert(0, "/opt/trn_rl_repo")

import concourse.bass as bass
import concourse.mybir as mybir
from concourse import tile
from concourse.bass_utils import run_bass_kernel_spmd
from concourse.vector_clock import ScopedClock


def _drain_and_barrier_single_wait(self, tick_clock, wait_clock):
    """TileContext._drain_and_barrier, but the kernel-tail drain's
    global-clock waits are spread over a chain of single-wait drains —
    the walrus build here allows only one sync wait per instruction."""
    drain_inst = self.nc.sync.drain()
    wait_clock.add_sem_waits(
        drain_inst.ins, ScopedClock({None: tick_clock.global_clock})
    )
    si = drain_inst.ins.sync_info
    waits = list(si.on_wait)
    if len(waits) > 1:
        si.on_wait = [waits[0]]
        sems = {h.name: h for h in self.sems.allocated().values()}
        for w in waits[1:]:
            d2 = self.nc.sync.drain()
            d2.wait_op(sems[w.ant_name], w.wait_value, "sem-ge")
    self.nc.all_engine_barrier()
    assert self.sems is not None
    popped = self.nc._tile_sem_poison_stack.pop()
    assert popped is self._sem_poison
    self.nc.clear_and_free_semaphores(list(self.sems.allocated().values()))
    self.nc.all_engine_barrier()


tile.TileContext._drain_and_barrier = _drain_and_barrier_single_wait

N_CORES = 8
S_TOTAL = 32          # number of scales
S_PER = 4             # scales per core: 2 small (1..16) + 2 large (17..32)
P = 128               # partition / block size
NT = 8192             # time samples
LMAX = 256            # padded kernel length
NBLK_U = 66           # unshifted input blocks: (128 + 8192 + 128) / 128
NBLK_S = 65           # +64-shifted input blocks
NGRP = 16             # groups of 4 output blocks (N=512 matmuls)
NCHUNK = 8            # x DMA chunks (2 groups each)
F32 = mybir.dt.float32
BF16 = mybir.dt.bfloat16
FP8 = mybir.dt.float8e4

WCOL = 2 * 256 + 2 * 384            # 1280 weight cols (2 small + 2 large)
# Only 8 DMAHW semaphore lanes exist; with 2 output DMAs that leaves 6
# input chunks (weights ride in chunk 0).  Chunk c covers CHUNK_GROUPS[c]
# output groups; its slab carries 2 (xu) / 1 (xs) lookahead blocks.
CHUNK_GROUPS = [2, 4, 4, 3, 3]
CHUNK_FIRST_GROUP = [0, 2, 6, 10, 13]
CHUNK_XU_BLK = [4 * g + 2 for g in CHUNK_GROUPS]
CHUNK_XS_BLK = [4 * g + 1 for g in CHUNK_GROUPS]
CHUNK_BASE = []
_col = WCOL
for _g, _u, _s in zip(CHUNK_GROUPS, CHUNK_XU_BLK, CHUNK_XS_BLK):
    CHUNK_BASE.append(_col)
    _col += (_u + _s) * P
NCOL = _col
NCHUNK = len(CHUNK_GROUPS)

GROUP_CHUNK = []                    # ng -> (chunk, within-chunk group idx)
for _c, (_f, _g) in enumerate(zip(CHUNK_FIRST_GROUP, CHUNK_GROUPS)):
    for _r in range(_g):
        GROUP_CHUNK.append((_c, _r))

LAST_RESULTS = None   # BassKernelResults of the most recent run (for test.py)


def _morlet_kernel_bank(n_scales: int, n: int) -> np.ndarray:
    Lmax = min(8 * n_scales, n)
    bank = np.zeros((n_scales, Lmax), dtype=np.float32)
    for i, s in enumerate(range(1, n_scales + 1)):
        L = min(8 * s, n)
        t = np.linspace(-4.0 * s, 4.0 * s, L)
        w = np.exp(-t**2 / (2.0 * s**2)) * np.cos(5.0 * t / s)
        w = w / np.sqrt(s)
        off = (Lmax - 1) // 2 - (L - 1) // 2
        bank[i, off : off + L] = w.astype(np.float32)
    return bank


def _core_scales(c: int) -> list[int]:
    """0-based scale indices handled by core c: 2 small then 2 large."""
    return [2 * c, 2 * c + 1, 16 + 2 * c, 16 + 2 * c + 1]


def _toeplitz_cols(gq: np.ndarray) -> np.ndarray:
    """Per-core weight columns [128, WCOL] from quantized reversed bank gq.

    Small scale i at cols [256i, 256i+256):  (p, j*128+to) = g'[64+128j+p-to]
    Large scale i at cols [512+384i, ...):   G2 (p, j*128+to) = g'[128j+p-to]
                                             G3 (p, 256+to)   = g'[256+p-to]
    """
    p = np.arange(P)[:, None]
    to = np.arange(P)[None, :]

    def gslice(row, d):
        v = np.zeros(d.shape, dtype=np.float32)
        ok = (d >= 0) & (d < LMAX)
        v[ok] = row[np.clip(d, 0, LMAX - 1)][ok]
        return v

    w = np.zeros((P, WCOL), dtype=np.float32)
    for i in range(2):          # small scales
        row = gq[i]
        base = 256 * i
        for j in range(2):
            w[:, base + 128 * j : base + 128 * (j + 1)] = gslice(
                row, 64 + 128 * j + p - to
            )
    for i in range(2):          # large scales
        row = gq[2 + i]
        base = 512 + 384 * i
        for j in range(2):
            w[:, base + 128 * j : base + 128 * (j + 1)] = gslice(
                row, 128 * j + p - to
            )
        w[:, base + 256 : base + 384] = gslice(row, 256 + p - to)
    return w


def _build_nc() -> bass.Bass:
    nc = bass.Bass()
    xg = nc.dram_tensor("xg", [P, NCOL], FP8, kind="ExternalInput")
    outp = nc.dram_tensor("outp", [1, S_PER * 512], F32, kind="ExternalOutput")

    with tile.TileContext(nc) as tc, ExitStack() as ctx:
        xpool = ctx.enter_context(tc.tile_pool(name="x", bufs=1))
        sqpool = ctx.enter_context(tc.tile_pool(name="sq", bufs=1))
        cppool = ctx.enter_context(tc.tile_pool(name="cp", bufs=4))
        rowpool = ctx.enter_context(tc.tile_pool(name="row", bufs=4))
        onepool = ctx.enter_context(tc.tile_pool(name="one", bufs=1))
        pspool = ctx.enter_context(tc.tile_pool(name="ps", bufs=4, space="PSUM"))
        psepool = ctx.enter_context(tc.tile_pool(name="pse", bufs=1, space="PSUM"))

        xgsb = xpool.tile([P, NCOL], FP8)
        # chunked input DMA in consumption order: weights first (small,
        # unblocks the HAM warmup matmuls early), then 5 x chunks.
        # 6 input + 2 output DMAs = all 8 DMAHW lanes, no recycling.
        nc.sync.dma_start(out=xgsb[:, :WCOL], in_=xg[:, :WCOL])
        for c in range(NCHUNK):
            lo = CHUNK_BASE[c]
            hi = lo + (CHUNK_XU_BLK[c] + CHUNK_XS_BLK[c]) * P
            nc.sync.dma_start(out=xgsb[:, lo:hi], in_=xg[:, lo:hi])

        onesb = onepool.tile([P, 32], FP8, tag="ones", name="ones")
        nc.vector.memset(onesb[:, :], 1.0)

        # warmup matmuls on the weights chunk: trip the HAM clock gate
        # while the first x slab is still in flight (no consumers)
        for wi in range(4):
            wt = pspool.tile([P, 512], F32, tag="conv")
            nc.tensor.matmul(
                wt[:, :], xgsb[:, 0:P], xgsb[:, 0:512], start=True, stop=True
            )

        # per-scale PSUM energy accumulators [1, (Bsub, b)]
        pes = [
            psepool.tile([1, 512], F32, tag=f"pe{s}", name=f"pe{s}")
            for s in range(S_PER)
        ]

        DR = mybir.MatmulPerfMode.DoubleRow
        ORDER = [0, 2, 1, 3]   # small, large, small, large
        sqtiles = {}           # (si, pair) -> fp8 pair tile [128, 1024]
        for ng in range(NGRP):
            c, r = GROUP_CHUNK[ng]
            ch_base = CHUNK_BASE[c]
            xu_base = ch_base + 4 * r * P
            xs_base = ch_base + CHUNK_XU_BLK[c] * P + 4 * r * P
            if r == 0:
                # guard matmul: carries the chunk-c DMA wait so the real
                # conv matmuls below never need a second sem wait
                gt = pspool.tile([P, 512], F32, tag="conv")
                nc.tensor.matmul(
                    gt[:, :64],
                    xgsb[:, ch_base : ch_base + P],
                    xgsb[:, ch_base : ch_base + 64],
                    start=True,
                    stop=True,
                )
            for pos, si in enumerate(ORDER):
                i = ng * S_PER + pos
                pt = pspool.tile([P, 512], F32, tag="conv")
                if si < 2:      # small scale: single DoubleRow pass
                    lhsT = xgsb[:, 256 * si : 256 * si + 256].rearrange(
                        "p (j t) -> p j t", j=2
                    )
                    nc.tensor.matmul(
                        pt[:, :], lhsT, _xwin(xgsb, xs_base), start=True,
                        stop=True, perf_mode=DR,
                    )
                else:           # large scale: DoubleRow + plain K=128 pass
                    base_w = 512 + 384 * (si - 2)
                    lhsT2 = xgsb[:, base_w : base_w + 256].rearrange(
                        "p (j t) -> p j t", j=2
                    )
                    nc.tensor.matmul(
                        pt[:, :], lhsT2, _xwin(xgsb, xu_base), start=True,
                        stop=False, perf_mode=DR,
                    )
                    lhsT3 = xgsb[:, base_w + 256 : base_w + 384]
                    rhs3 = xgsb[:, xu_base + 2 * P : xu_base + 6 * P]
                    nc.tensor.matmul(
                        pt[:, :], lhsT3, rhs3, start=False, stop=True,
                    )
                # squares land in fp8 pair tiles [128, (half, 512)]; one
                # DoubleRow reduce per pair halves the reduce matmul count.
                # Both halves of a pair use the SAME engine (single wait).
                pair = ng // 2
                half = ng % 2
                pidx = pair * S_PER + si
                if half == 0:
                    sqtiles[(si, pair)] = sqpool.tile(
                        [P, 1024], FP8, name=f"sq{si}_{pair}"
                    )
                sq = sqtiles[(si, pair)]
                dst = sq[:, half * 512 : half * 512 + 512]
                if pidx % 8 not in (1, 4, 7):
                    # ACT path: square+cast straight out of PSUM (20/32)
                    nc.scalar.square(dst, pt[:, :])
                else:
                    # DVE path: bf16 copy out of PSUM, then square (12/32)
                    cp = cppool.tile([P, 512], BF16)
                    nc.vector.tensor_copy(cp[:, :], pt[:, :])
                    nc.vector.tensor_mul(dst, cp[:, :], cp[:, :])
                if half == 1:
                    ones_dr = bass.AP(
                        onesb.tensor, onesb[:, :].offset,
                        [list(onesb[:, :].ap[0]), [16, 2], [1, 1]],
                    )
                    nc.tensor.matmul(
                        pes[si][:, :],
                        ones_dr,
                        sq[:, :].rearrange("p (j n) -> p j n", j=2),
                        start=(ng == 1),
                        stop=(ng == NGRP - 1),
                        perf_mode=DR,
                    )

        # final: evict the [1,512] accumulators side by side on partition 0.
        # ACT evicts scales {0,2} into the low half, DVE evicts {1,3} into
        # the high half; each half goes out via its own DMA so every DMA
        # carries exactly one engine sem wait (host un-permutes).
        rowout = rowpool.tile([1, S_PER * 512], F32, tag="rowout", name="rowout")
        nc.scalar.copy(rowout[:, 0:512], pes[0][:, :])
        nc.scalar.copy(rowout[:, 512:1024], pes[2][:, :])
        nc.vector.tensor_copy(rowout[:, 1024:1536], pes[1][:, :])
        nc.vector.tensor_copy(rowout[:, 1536:2048], pes[3][:, :])
        # Activation-engine DMA queue: the SP queue's ring credits are
        # used up by the 9 input chunks (a 10th SP DMA would carry a
        # ring-credit wait on top of its data dep = 2 waits)
        nc.scalar.dma_start(out=outp[:, 0:1024], in_=rowout[:, 0:1024])
        nc.scalar.dma_start(out=outp[:, 1024:2048], in_=rowout[:, 1024:2048])

    return nc


def _xwin(xgsb, base):
    """rhs AP [128, j=2 (stride 128 cols, overlapping), 512] at col base."""
    sl = xgsb[:, base : base + 5 * P]
    return bass.AP(sl.tensor, sl.offset, [list(sl.ap[0]), [P, 2], [1, 512]])


_NC_CACHE = None


def _get_nc() -> bass.Bass:
    global _NC_CACHE
    if _NC_CACHE is None:
        _NC_CACHE = _build_nc()
    return _NC_CACHE


def kernel(x: np.ndarray, scale_weights: np.ndarray, _trace: bool = False) -> np.ndarray:
    global LAST_RESULTS
    import ml_dtypes

    e4 = ml_dtypes.float8_e4m3fn
    x = np.asarray(x, dtype=np.float32)
    scale_weights = np.asarray(scale_weights, dtype=np.float32)
    assert x.shape == (P, NT) and scale_weights.shape == (S_TOTAL,)

    bank = _morlet_kernel_bank(S_TOTAL, NT)          # [32, 256] fp32
    gq = bank[:, ::-1].astype(e4).astype(np.float32)  # quantized g' rows

    xq8 = x.T.astype(e4)                              # [NT, P] fp8
    xq = xq8.astype(np.float32)

    # time-major blocked layouts (fp8 bytes)
    xpad = np.zeros((NBLK_U * P, P), dtype=e4)
    xpad[P : P + NT, :] = xq8
    xb_u = xpad.reshape(NBLK_U, P, P).transpose(1, 0, 2).reshape(P, NBLK_U * P)
    xsh = xpad[64 : 64 + NBLK_S * P, :]
    xb_s = np.ascontiguousarray(xsh).reshape(NBLK_S, P, P).transpose(1, 0, 2).reshape(
        P, NBLK_S * P
    )

    xgs = []
    for c in range(N_CORES):
        scales = _core_scales(c)
        w = _toeplitz_cols(gq[scales]).astype(e4)     # [128, WCOL]
        buf = np.empty((P, NCOL), dtype=e4)
        buf[:, :WCOL] = w
        for ch in range(NCHUNK):
            lo = CHUNK_BASE[ch]
            u0 = 4 * CHUNK_FIRST_GROUP[ch] * P
            nu, ns = CHUNK_XU_BLK[ch] * P, CHUNK_XS_BLK[ch] * P
            buf[:, lo : lo + nu] = xb_u[:, u0 : u0 + nu]
            buf[:, lo + nu : lo + nu + ns] = xb_s[:, u0 : u0 + ns]
        xgs.append(buf)

    nc = _get_nc()
    in_maps = [{"xg": xgs[c]} for c in range(N_CORES)]
    res = run_bass_kernel_spmd(nc, in_maps, list(range(N_CORES)), trace=_trace)
    LAST_RESULTS = res

    # gather + unshard: core c covers scale ids [2c, 2c+1, 16+2c, 17+2c]
    esum = np.zeros((S_TOTAL, P), dtype=np.float64)
    for c in range(N_CORES):
        vals = res.results[c]["outp"].reshape(S_PER, 4, P).sum(axis=1)
        sc = _core_scales(c)
        # device row order is [si=0, 2, 1, 3] (ACT low half, DVE high half)
        for row, si in enumerate([0, 2, 1, 3]):
            esum[sc[si]] = vals[row]
    energy = esum.T / np.float64(NT)                  # [128 b, 32 s]

    # exact correction of the deterministic fp8 norm bias
    w2 = (bank.astype(np.float64) ** 2).sum(1)        # [32]
    wq2 = (gq.astype(np.float64) ** 2).sum(1)
    mx2 = (x.astype(np.float64) ** 2).mean(1)         # [128]
    mxq2 = (xq.T.astype(np.float64) ** 2).mean(1)
    energy = energy * (mx2[:, None] * w2[None, :]) / (mxq2[:, None] * wq2[None, :])

    w = scale_weights.astype(np.float64)
    e = np.exp(w - w.max())
    sm = e / e.sum()
    return (energy * sm[None, :]).astype(np.float32)


if __name__ == "__main__":
    rng = np.random.default_rng(0)
    x = rng.standard_normal((P, NT), dtype=np.float32)
    sw = rng.standard_normal(S_TOTAL, dtype=np.float32)
    out = kernel(x, sw)
    print("kernel output shape:", out.shape, out.dtype)



# revision 5
# speedup vs baseline: 1.4420x; 1.4420x over previous
"""Trainium2 Bass kernel for ContinuousWaveletLayer (CWT energy), v3.

Reference computation:
  bank = Morlet wavelet bank [32 scales, Lmax=256] (static)
  coef[b,s,t] = 'same' conv of x[b,:] (len 8192) with bank[s,:]
  out[b,s]    = mean_t(coef^2) * softmax(scale_weights)[s]

v3 strategy (vs v2's 54.5us):
  * Morlet coefficients at scale s are band-limited (center 5/s rad,
    Gaussian width ~1/s), so mean_t(coef^2) can be estimated from a
    stride-4 time subsample (x4) for s >= 9 with < 1e-3 aliasing error
    (validated numerically; s=9 is 2.5e-3).  This cuts a scale's PE
    cost from 8192/16384 streamed columns to 6144 and its square /
    reduce cost 4x.
  * Uniform SPMD shape: every core runs 1 exact small scale (1..8) +
    3 strided scales; all 65 small-scale out-blocks are offset by +64
    so a single K=256 DoubleRow window covers the kernel support with
    UNSHIFTED x (edge out-blocks use partial-partition squares instead
    of masked weights); the +64-shifted x copy of v2 is gone, halving
    input DMA to 1.38MB/core.
  * Strided conv matmuls use 4-level rhs APs [p][j][m'][b] so one
    N=512 matmul covers 4 decimated out-blocks.
  * fp8(e4m3) everywhere; DR reduces over fp8 squares; deterministic
    fp8 norm bias divided out exactly on the host (as in v2).
"""

import sys
from contextlib import ExitStack

import numpy as np

sys.path.insert(0, "/opt/trn_rl_repo")

import concourse.bass as bass
import concourse.mybir as mybir
from concourse import tile
from concourse.bass_utils import run_bass_kernel_spmd
from concourse.vector_clock import ScopedClock


def _drain_and_barrier_single_wait(self, tick_clock, wait_clock):
    """TileContext._drain_and_barrier, but the kernel-tail drain's
    global-clock waits are spread over a chain of single-wait drains —
    the walrus build here allows only one sync wait per instruction."""
    drain_inst = self.nc.sync.drain()
    wait_clock.add_sem_waits(
        drain_inst.ins, ScopedClock({None: tick_clock.global_clock})
    )
    si = drain_inst.ins.sync_info
    waits = list(si.on_wait)
    if len(waits) > 1:
        si.on_wait = [waits[0]]
        sems = {h.name: h for h in self.sems.allocated().values()}
        for w in waits[1:]:
            d2 = self.nc.sync.drain()
            d2.wait_op(sems[w.ant_name], w.wait_value, "sem-ge")
    self.nc.all_engine_barrier()
    assert self.sems is not None
    popped = self.nc._tile_sem_poison_stack.pop()
    assert popped is self._sem_poison
    self.nc.clear_and_free_semaphores(list(self.sems.allocated().values()))
    self.nc.all_engine_barrier()


tile.TileContext._drain_and_barrier = _drain_and_barrier_single_wait

N_CORES = 8
S_TOTAL = 32
P = 128
NT = 8192
LMAX = 256
NBLK = 66             # x blocks incl. 1 pad block each side
F32 = mybir.dt.float32
BF16 = mybir.dt.bfloat16
FP8 = mybir.dt.float8e4
DR = mybir.MatmulPerfMode.DoubleRow

# weights region: small-scale lhsT (256 cols) + 3 strided scales (3x256)
W_NS = 0                      # small-scale lhsT [128, (j2,128)]
W_S4 = [256, 1024, 1792]      # strided lhsT base (3 passes x 256 each)
WCOL = 2560
XB = WCOL                     # x region base col
NCOL = WCOL + NBLK * P        # 11008

# input DMA chunks (col ranges), consumption-ordered on one queue:
#   weights | x blocks 0..6 | 7..18 | 19..34 | 35..50 | 51..65
CHUNKS = [
    (0, WCOL),
    (XB, XB + 7 * P),
    (XB + 7 * P, XB + 19 * P),
    (XB + 19 * P, XB + 35 * P),
    (XB + 35 * P, XB + 51 * P),
    (XB + 51 * P, XB + NBLK * P),
]

# square-engine map: True = ACT (1-op square), False = DVE (copy+mul).
# Regions are 1024-col DR reduce pairs; both halves of a pair must be
# written by the SAME engine (single sync wait on the reduce).
NS_PAIR_ACT = [False, True, True, False, True, True, True, True]  # r0..r7
NS_TAIL_ACT = False
S4_PAIR_ACT = [[False, True], [True, True], [True, False]]  # [scale][pair]

LAST_RESULTS = None


def _morlet_kernel_bank(n_scales: int, n: int) -> np.ndarray:
    Lmax = min(8 * n_scales, n)
    bank = np.zeros((n_scales, Lmax), dtype=np.float32)
    for i, s in enumerate(range(1, n_scales + 1)):
        L = min(8 * s, n)
        t = np.linspace(-4.0 * s, 4.0 * s, L)
        w = np.exp(-t**2 / (2.0 * s**2)) * np.cos(5.0 * t / s)
        w = w / np.sqrt(s)
        off = (Lmax - 1) // 2 - (L - 1) // 2
        bank[i, off : off + L] = w.astype(np.float32)
    return bank


def _core_scales(c: int) -> list[int]:
    """0-based scale ids on core c: [small, s4a, s4b, s4c]."""
    return [c, 8 + 3 * c, 9 + 3 * c, 10 + 3 * c]


def _gslice(row, idx):
    v = np.zeros(idx.shape, dtype=np.float32)
    ok = (idx >= 0) & (idx < LMAX)
    v[ok] = row[np.clip(idx, 0, LMAX - 1)][ok]
    return v


def _lhsT_small(gq: np.ndarray) -> np.ndarray:
    """[128, 256] cols (j, to): w[p, 128j+to] = g'[128j + p - to + 63]."""
    p = np.arange(P)[:, None]
    to = np.arange(P)[None, :]
    return np.concatenate(
        [_gslice(gq, 128 * j + p - to + 63) for j in (0, 1)], axis=1
    )


def _lhsT_s4(gq: np.ndarray, q: int) -> np.ndarray:
    """stride-4 pass q: w[p, 128j+to] = g'[256q + 128j + p - 4to - 4]."""
    p = np.arange(P)[:, None]
    to = np.arange(P)[None, :]
    return np.concatenate(
        [_gslice(gq, 256 * q + 128 * j + p - 4 * to - 4) for j in (0, 1)],
        axis=1,
    )


def _xw4(xgsb, base_col, mstride):
    """4-level rhs AP [p][j:128,2][m:mstride,4][b:1,128] at base_col."""
    sl = xgsb[:, base_col : base_col + P]
    return bass.AP(
        sl.tensor, sl.offset, [list(sl.ap[0]), [P, 2], [mstride, 4], [1, P]]
    )


def _xw3(xgsb, base_col, n):
    """3-level rhs AP [p][j:128,2][n:1,n] at base_col."""
    sl = xgsb[:, base_col : base_col + P]
    return bass.AP(sl.tensor, sl.offset, [list(sl.ap[0]), [P, 2], [1, n]])


def _pair_ap(sq, lo, jstride, n):
    """DR reduce rhs [p][j:jstride,2][n:1,n] over sq fp8 tile at col lo."""
    sl = sq[:, lo : lo + n]
    return bass.AP(sl.tensor, sl.offset, [list(sl.ap[0]), [jstride, 2], [1, n]])


def _build_nc() -> bass.Bass:
    nc = bass.Bass()
    xg = nc.dram_tensor("xg", [P, NCOL], FP8, kind="ExternalInput")
    outp = nc.dram_tensor("outp", [1, 4 * 512], F32, kind="ExternalOutput")

    with tile.TileContext(nc) as tc, ExitStack() as ctx:
        xpool = ctx.enter_context(tc.tile_pool(name="x", bufs=1))
        sqpool = ctx.enter_context(tc.tile_pool(name="sq", bufs=1))
        cppool = ctx.enter_context(tc.tile_pool(name="cp", bufs=4))
        rowpool = ctx.enter_context(tc.tile_pool(name="row", bufs=1))
        pspool = ctx.enter_context(tc.tile_pool(name="ps", bufs=4, space="PSUM"))
        psepool = ctx.enter_context(tc.tile_pool(name="pse", bufs=1, space="PSUM"))

        xgsb = xpool.tile([P, NCOL], FP8)
        scr = xpool.tile([P, 512], FP8, name="scr")     # warmup scratch
        ones = xpool.tile([P, 32], FP8, name="ones")
        sqNS = sqpool.tile([P, 8320], FP8, name="sqNS")
        sqS4 = [sqpool.tile([P, 2048], FP8, name=f"sqS4_{k}") for k in range(3)]
        rowout = rowpool.tile([1, 4 * 512], F32, name="rowout")

        # input DMA chunks, consumption-ordered on the SP queue
        for lo, hi in CHUNKS:
            nc.sync.dma_start(out=xgsb[:, lo:hi], in_=xg[:, lo:hi])

        # DVE init: warmup scratch + ones + sq edge zeros (the small-scale
        # edge out-blocks write only half their partitions; the other half
        # must read as zero in the reduce)
        nc.vector.memset(scr[:, :], 1.0)
        nc.vector.memset(ones[:, :], 1.0)
        nc.vector.memset(sqNS[0:64, 0:128], 0.0)
        nc.vector.memset(sqNS[64:128, 8192:8320], 0.0)

        # acc bank: rows 0 of four 512-col regions hold the per-scale
        # energy accumulators; warmup/guard matmuls write the (otherwise
        # unused) full-partition region before any reduce starts
        accbank = psepool.tile([P, 4 * 512], F32, name="accbank")

        # warmup matmuls on DVE-initialized scratch: ramp the PE clock
        # while input DMA is still in flight (no DMA dependency at all)
        for _ in range(4):
            nc.tensor.matmul(
                accbank[:, 0:512], scr[:, 0:128], scr[:, :], start=True, stop=True,
                skip_group_check=True,
            )

        # guard matmuls: one per input chunk; each carries that chunk's DMA
        # sem wait so real matmuls below never need a second wait
        def guard(chunk_idx):
            # writes partitions 64.. only: partition 0 holds the live
            # energy accumulators, which guards must not reset
            lo = CHUNKS[chunk_idx][0]
            nc.tensor.matmul(
                accbank[64:128, 0:64], xgsb[:, lo : lo + 64], xgsb[:, lo : lo + 64],
                start=True, stop=True, skip_group_check=True,
            )

        # per-scale PSUM energy accumulator views [1, 512]
        accs = [accbank[0:1, 512 * i : 512 * i + 512] for i in range(4)]
        acc_started = [False] * 4
        acc_nred = [9, 2, 2, 2]      # reduces per acc
        acc_done = [0] * 4

        lhsT_NS = xgsb[:, W_NS : W_NS + 256].rearrange("p (j t) -> p j t", j=2)
        lhsT_S4 = [
            [
                xgsb[:, W_S4[k] + 256 * q : W_S4[k] + 256 * q + 256].rearrange(
                    "p (j t) -> p j t", j=2
                )
                for q in range(3)
            ]
            for k in range(3)
        ]
        ones_dr = bass.AP(
            ones.tensor, ones[:, :].offset, [list(ones[:, :].ap[0]), [16, 2], [1, 1]]
        )

        def square(eng_act, dst_sq, lo, n, pt, plo=0, phi=P):
            """square psum [plo:phi, 0:n] into dst_sq[plo:phi, lo:lo+n]."""
            if eng_act:
                nc.scalar.square(dst_sq[plo:phi, lo : lo + n], pt[plo:phi, 0:n])
            else:
                cp = cppool.tile([P, 512], BF16)
                nc.vector.tensor_copy(cp[plo:phi, 0:n], pt[plo:phi, 0:n])
                nc.vector.tensor_mul(
                    dst_sq[plo:phi, lo : lo + n], cp[plo:phi, 0:n], cp[plo:phi, 0:n]
                )

        def reduce(ai, rhs_ap, n, dr=True):
            acc_done[ai] += 1
            nc.tensor.matmul(
                accs[ai][:, 0:n], ones_dr if dr else ones[:, 0:1], rhs_ap,
                start=not acc_started[ai], stop=acc_done[ai] == acc_nred[ai],
                perf_mode=DR if dr else None, skip_group_check=True,
            )
            acc_started[ai] = True

        # ---- main schedule: 4 quarters ----
        guard(0)   # weights
        guard(1)   # x blocks 0..6
        for T in range(4):
            # small scale: groups g = 4T..4T+3, out-blocks m = 4g-1..4g+2,
            # rhs base col = XB + 128*4g
            for i in range(4):
                g = 4 * T + i
                if T == 0 and i == 1:
                    guard(2)      # x blocks 7..18 (needed from g1 on)
                pt = pspool.tile([P, 512], F32, tag="conv")
                nc.tensor.matmul(
                    pt[:, :], lhsT_NS, _xw4(xgsb, XB + 512 * g, P),
                    start=True, stop=True, perf_mode=DR,
                )
                act = NS_PAIR_ACT[g // 2]
                if g == 0:
                    # m=-1 edge: only out-times 0..63 (partitions 64:) valid
                    square(act, sqNS, 0, 128, pt, plo=64)
                    sl = sqNS[:, 128:512]
                    if act:
                        nc.scalar.square(sl, pt[:, 128:512])
                    else:
                        cp = cppool.tile([P, 512], BF16)
                        nc.vector.tensor_copy(cp[:, 0:384], pt[:, 128:512])
                        nc.vector.tensor_mul(sl, cp[:, 0:384], cp[:, 0:384])
                else:
                    square(act, sqNS, 512 * g, 512, pt)
            # strided scales: m' group = 4T..4T+3, 3 accumulated passes
            for k in range(3):
                pt = pspool.tile([P, 512], F32, tag="conv")
                for q in range(3):
                    nc.tensor.matmul(
                        pt[:, :], lhsT_S4[k][q],
                        _xw4(xgsb, XB + P * (16 * T + 2 * q), 4 * P),
                        start=q == 0, stop=q == 2, perf_mode=DR,
                    )
                square(S4_PAIR_ACT[k][T // 2], sqS4[k], 512 * T, 512, pt)

            # interleaved reduces + next chunk guards
            if T == 0:
                guard(3)                       # x blocks 19..34
                reduce(0, _pair_ap(sqNS, 0, 512, 512), 512)          # r0 DVE
                reduce(0, _pair_ap(sqNS, 1024, 512, 512), 512)       # r1 ACT
            elif T == 1:
                guard(4)                       # x blocks 35..50
                reduce(0, _pair_ap(sqNS, 2048, 512, 512), 512)       # r2
                reduce(0, _pair_ap(sqNS, 3072, 512, 512), 512)       # r3
                for k in range(3):             # S4 pair 0 (quarters 0+1)
                    reduce(1 + k, _pair_ap(sqS4[k], 0, 512, 512), 512)
            elif T == 2:
                guard(5)                       # x blocks 51..65
                reduce(0, _pair_ap(sqNS, 4096, 512, 512), 512)       # r4
                reduce(0, _pair_ap(sqNS, 5120, 512, 512), 512)       # r5

        # tail: small-scale out-block m=63 (N=128, out-times 0..63 valid)
        pt = pspool.tile([P, 512], F32, tag="conv")
        nc.tensor.matmul(
            pt[:, 0:128], lhsT_NS, _xw3(xgsb, XB + 512 * 16, 128),
            start=True, stop=True, perf_mode=DR,
        )
        square(NS_TAIL_ACT, sqNS, 8192, 128, pt, phi=64)

        # remaining reduces
        reduce(0, _pair_ap(sqNS, 6144, 512, 512), 512)               # r6
        reduce(0, _pair_ap(sqNS, 7168, 512, 512), 512)               # r7
        reduce(0, sqNS[:, 8192:8320], 128, dr=False)                 # tail
        for k in range(3):                     # S4 pair 1 (quarters 2+3)
            reduce(1 + k, _pair_ap(sqS4[k], 1024, 512, 512), 512)

        # evict: ACT copies accs 0,1; DVE copies accs 2,3; two output DMAs
        nc.scalar.copy(rowout[:, 0:1024], accbank[0:1, 0:1024])
        nc.vector.tensor_copy(rowout[:, 1024:2048], accbank[0:1, 1024:2048])
        nc.sync.dma_start(out=outp[:, 0:1024], in_=rowout[:, 0:1024])
        nc.scalar.dma_start(out=outp[:, 1024:2048], in_=rowout[:, 1024:2048])

    return nc


def _strip_pe_self_waits(nc: bass.Bass):
    """Drop PE-on-PE semaphore waits.  The PE executes its stream in
    order, so a WAW between two PE matmuls (psum buffer recycling) never
    needs a semaphore; the tile scheduler occasionally emits one anyway,
    which trips the walrus single-wait limit."""
    for blk in nc.m.functions[0].blocks:
        for ins in blk.instructions:
            si = getattr(ins, "sync_info", None)
            if si is None:
                continue
            waits = list(si.on_wait)
            if len(waits) <= 1:
                continue
            if ins.engine == mybir.EngineType.PE:
                keep = [w for w in waits if not w.ant_name.startswith("PE_")]
                if len(keep) < len(waits) and len(keep) <= 1:
                    si.on_wait = keep
    for blk in nc.m.functions[0].blocks:
        for ins in blk.instructions:
            si = getattr(ins, "sync_info", None)
            if si is not None and len(list(si.on_wait)) > 1:
                raise RuntimeError(f"multi-wait survives: {ins.name}")


_NC_CACHE = None


def _get_nc() -> bass.Bass:
    global _NC_CACHE
    if _NC_CACHE is None:
        _NC_CACHE = _build_nc()
        _strip_pe_self_waits(_NC_CACHE)
    return _NC_CACHE


def kernel(x: np.ndarray, scale_weights: np.ndarray, _trace: bool = False) -> np.ndarray:
    global LAST_RESULTS
    import ml_dtypes

    e4 = ml_dtypes.float8_e4m3fn
    x = np.asarray(x, dtype=np.float32)
    scale_weights = np.asarray(scale_weights, dtype=np.float32)
    assert x.shape == (P, NT) and scale_weights.shape == (S_TOTAL,)

    bank = _morlet_kernel_bank(S_TOTAL, NT)           # [32, 256] fp32
    gq = bank[:, ::-1].astype(e4).astype(np.float32)  # quantized g' rows

    xq8 = x.T.astype(e4)                              # [NT, P] fp8
    # x layout: xcol[p, 128*I + b] = xpad[128*I + p, b]
    xpad = np.zeros((NBLK * P, P), dtype=e4)
    xpad[P : P + NT, :] = xq8
    xcol = xpad.reshape(NBLK, P, P).transpose(1, 0, 2).reshape(P, NBLK * P)

    xgs = []
    for c in range(N_CORES):
        sc = _core_scales(c)
        buf = np.empty((P, NCOL), dtype=e4)
        buf[:, W_NS : W_NS + 256] = _lhsT_small(gq[sc[0]]).astype(e4)
        for k in range(3):
            for q in range(3):
                buf[:, W_S4[k] + 256 * q : W_S4[k] + 256 * q + 256] = _lhsT_s4(
                    gq[sc[1 + k]], q
                ).astype(e4)
        buf[:, XB:] = xcol
        xgs.append(buf)

    nc = _get_nc()
    in_maps = [{"xg": xgs[c]} for c in range(N_CORES)]
    res = run_bass_kernel_spmd(nc, in_maps, list(range(N_CORES)), trace=_trace)
    LAST_RESULTS = res

    # gather: core c rows = [small scale c, 8+3c, 9+3c, 10+3c]
    energy = np.zeros((P, S_TOTAL), dtype=np.float64)
    for c in range(N_CORES):
        vals = res.results[c]["outp"].reshape(4, 4, P).astype(np.float64).sum(axis=1)
        sc = _core_scales(c)
        energy[:, sc[0]] = vals[0] / NT
        for k in range(3):
            energy[:, sc[1 + k]] = vals[1 + k] * 4.0 / NT

    # exact correction of the deterministic fp8 norm bias
    w2 = (bank.astype(np.float64) ** 2).sum(1)
    wq2 = (gq.astype(np.float64) ** 2).sum(1)
    mx2 = (x.astype(np.float64) ** 2).mean(1)
    mxq2 = (xq8.T.astype(np.float64) ** 2).mean(1)
    energy = energy * (mx2[:, None] * w2[None, :]) / (mxq2[:, None] * wq2[None, :])

    w = scale_weights.astype(np.float64)
    e = np.exp(w - w.max())
    sm = e / e.sum()
    return (energy * sm[None, :]).astype(np.float32)


if __name__ == "__main__":
    rng = np.random.default_rng(0)
    x = rng.standard_normal((P, NT), dtype=np.float32)
    sw = rng.standard_normal(S_TOTAL, dtype=np.float32)
    out = kernel(x, sw)
    print("kernel output shape:", out.shape, out.dtype)


# revision 7
# speedup vs baseline: 1.5137x; 1.0497x over previous
"""Trainium2 Bass kernel for ContinuousWaveletLayer (CWT energy), v3.

Reference computation:
  bank = Morlet wavelet bank [32 scales, Lmax=256] (static)
  coef[b,s,t] = 'same' conv of x[b,:] (len 8192) with bank[s,:]
  out[b,s]    = mean_t(coef^2) * softmax(scale_weights)[s]

v3 strategy (vs v2's 54.5us):
  * Morlet coefficients at scale s are band-limited (center 5/s rad,
    Gaussian width ~1/s), so mean_t(coef^2) can be estimated from a
    stride-4 time subsample (x4) for s >= 9 with < 1e-3 aliasing error
    (validated numerically; s=9 is 2.5e-3).  This cuts a scale's PE
    cost from 8192/16384 streamed columns to 6144 and its square /
    reduce cost 4x.
  * Uniform SPMD shape: every core runs 1 exact small scale (1..8) +
    3 strided scales; all 65 small-scale out-blocks are offset by +64
    so a single K=256 DoubleRow window covers the kernel support with
    UNSHIFTED x (edge out-blocks use partial-partition squares instead
    of masked weights); the +64-shifted x copy of v2 is gone, halving
    input DMA to 1.38MB/core.
  * Strided conv matmuls use 4-level rhs APs [p][j][m'][b] so one
    N=512 matmul covers 4 decimated out-blocks.
  * fp8(e4m3) everywhere; DR reduces over fp8 squares; deterministic
    fp8 norm bias divided out exactly on the host (as in v2).
"""

import sys
from contextlib import ExitStack

import numpy as np

sys.path.insert(0, "/opt/trn_rl_repo")

import concourse.bass as bass
import concourse.mybir as mybir
from concourse import tile
from concourse.bass_utils import run_bass_kernel_spmd
from concourse.vector_clock import ScopedClock


def _drain_and_barrier_single_wait(self, tick_clock, wait_clock):
    """TileContext._drain_and_barrier, but the kernel-tail drain's
    global-clock waits are spread over a chain of single-wait drains —
    the walrus build here allows only one sync wait per instruction."""
    drain_inst = self.nc.sync.drain()
    wait_clock.add_sem_waits(
        drain_inst.ins, ScopedClock({None: tick_clock.global_clock})
    )
    si = drain_inst.ins.sync_info
    waits = list(si.on_wait)
    if len(waits) > 1:
        si.on_wait = [waits[0]]
        sems = {h.name: h for h in self.sems.allocated().values()}
        for w in waits[1:]:
            d2 = self.nc.sync.drain()
            d2.wait_op(sems[w.ant_name], w.wait_value, "sem-ge")
    self.nc.all_engine_barrier()
    assert self.sems is not None
    popped = self.nc._tile_sem_poison_stack.pop()
    assert popped is self._sem_poison
    self.nc.clear_and_free_semaphores(list(self.sems.allocated().values()))
    self.nc.all_engine_barrier()


tile.TileContext._drain_and_barrier = _drain_and_barrier_single_wait

N_CORES = 8
S_TOTAL = 32
P = 128
NT = 8192
LMAX = 256
NBLK = 66             # x blocks incl. 1 pad block each side
F32 = mybir.dt.float32
BF16 = mybir.dt.bfloat16
FP8 = mybir.dt.float8e4
DR = mybir.MatmulPerfMode.DoubleRow

# weights region: small-scale lhsT (256 cols) + 3 strided scales (3x256)
W_NS = 0                      # small-scale lhsT [128, (j2,128)]
W_S4 = [256, 1024, 1792]      # strided lhsT base (3 passes x 256 each)
WCOL = 2560
XB = WCOL                     # x region base col
NCOL = WCOL + NBLK * P        # 11008

# input DMA chunks (col ranges, queue): consumption-ordered; the two
# early x chunks ride the ACT hwdge queue so their transfers overlap the
# weights chunks on the SP queue.
CHUNKS = [
    (0, 256, "sp"),                            # small-scale lhsT
    (XB, XB + 10 * P, "act"),                  # x blocks 0..9
    (256, WCOL, "sp"),                         # strided lhsT
    (XB + 10 * P, XB + 19 * P, "act"),         # x blocks 10..18
    (XB + 19 * P, XB + 35 * P, "sp"),          # x blocks 19..34
    (XB + 35 * P, XB + NBLK * P, "sp"),        # x blocks 35..65
]

# square-engine map: True = ACT (1-op square), False = DVE (copy+mul).
# Regions are 1024-col DR reduce pairs; both halves of a pair must be
# written by the SAME engine (single sync wait on the reduce).
NS_PAIR_ACT = [False, True, True, False, True, True, True, True]  # r0..r7
NS_TAIL_ACT = False
S4_PAIR_ACT = [[False, True], [True, True], [False, True]]  # [scale][pair]

LAST_RESULTS = None


def _morlet_kernel_bank(n_scales: int, n: int) -> np.ndarray:
    Lmax = min(8 * n_scales, n)
    bank = np.zeros((n_scales, Lmax), dtype=np.float32)
    for i, s in enumerate(range(1, n_scales + 1)):
        L = min(8 * s, n)
        t = np.linspace(-4.0 * s, 4.0 * s, L)
        w = np.exp(-t**2 / (2.0 * s**2)) * np.cos(5.0 * t / s)
        w = w / np.sqrt(s)
        off = (Lmax - 1) // 2 - (L - 1) // 2
        bank[i, off : off + L] = w.astype(np.float32)
    return bank


def _core_scales(c: int) -> list[int]:
    """0-based scale ids on core c: [small, s4a, s4b, s4c]."""
    return [c, 8 + 3 * c, 9 + 3 * c, 10 + 3 * c]


def _gslice(row, idx):
    v = np.zeros(idx.shape, dtype=np.float32)
    ok = (idx >= 0) & (idx < LMAX)
    v[ok] = row[np.clip(idx, 0, LMAX - 1)][ok]
    return v


def _lhsT_small(gq: np.ndarray) -> np.ndarray:
    """[128, 256] cols (j, to): w[p, 128j+to] = g'[128j + p - to + 63]."""
    p = np.arange(P)[:, None]
    to = np.arange(P)[None, :]
    return np.concatenate(
        [_gslice(gq, 128 * j + p - to + 63) for j in (0, 1)], axis=1
    )


def _lhsT_s4(gq: np.ndarray, q: int) -> np.ndarray:
    """stride-4 pass q: w[p, 128j+to] = g'[256q + 128j + p - 4to - 4]."""
    p = np.arange(P)[:, None]
    to = np.arange(P)[None, :]
    return np.concatenate(
        [_gslice(gq, 256 * q + 128 * j + p - 4 * to - 4) for j in (0, 1)],
        axis=1,
    )


def _xw4(xgsb, base_col, mstride):
    """4-level rhs AP [p][j:128,2][m:mstride,4][b:1,128] at base_col."""
    sl = xgsb[:, base_col : base_col + P]
    return bass.AP(
        sl.tensor, sl.offset, [list(sl.ap[0]), [P, 2], [mstride, 4], [1, P]]
    )


def _xw3(xgsb, base_col, n):
    """3-level rhs AP [p][j:128,2][n:1,n] at base_col."""
    sl = xgsb[:, base_col : base_col + P]
    return bass.AP(sl.tensor, sl.offset, [list(sl.ap[0]), [P, 2], [1, n]])


def _pair_ap(sq, lo, jstride, n):
    """DR reduce rhs [p][j:jstride,2][n:1,n] over sq fp8 tile at col lo."""
    sl = sq[:, lo : lo + n]
    return bass.AP(sl.tensor, sl.offset, [list(sl.ap[0]), [jstride, 2], [1, n]])


def _build_nc() -> bass.Bass:
    nc = bass.Bass()
    xg = nc.dram_tensor("xg", [P, NCOL], FP8, kind="ExternalInput")
    outp = nc.dram_tensor("outp", [1, 4 * 512], F32, kind="ExternalOutput")

    with tile.TileContext(nc) as tc, ExitStack() as ctx:
        xpool = ctx.enter_context(tc.tile_pool(name="x", bufs=1))
        sqpool = ctx.enter_context(tc.tile_pool(name="sq", bufs=1))
        cppool = ctx.enter_context(tc.tile_pool(name="cp", bufs=4))
        rowpool = ctx.enter_context(tc.tile_pool(name="row", bufs=1))
        pspool = ctx.enter_context(tc.tile_pool(name="ps", bufs=4, space="PSUM"))
        psepool = ctx.enter_context(tc.tile_pool(name="pse", bufs=1, space="PSUM"))

        xgsb = xpool.tile([P, NCOL], FP8)
        scr = xpool.tile([P, 512], FP8, name="scr")     # warmup scratch
        ones = xpool.tile([P, 32], FP8, name="ones")
        sqNS = sqpool.tile([P, 8320], FP8, name="sqNS")
        sqS4 = [sqpool.tile([P, 2048], FP8, name=f"sqS4_{k}") for k in range(3)]
        rowout = rowpool.tile([1, 4 * 512], F32, name="rowout")

        # input DMA chunks on two hwdge queues
        for lo, hi, q in CHUNKS:
            eng = nc.sync if q == "sp" else nc.scalar
            eng.dma_start(out=xgsb[:, lo:hi], in_=xg[:, lo:hi])

        # DVE init: warmup scratch first (it gates the PE warmups), then
        # ones + sq edge zeros (the small-scale edge out-blocks write only
        # half their partitions; the other half must read 0 in the reduce)
        nc.vector.memset(scr[:, :], 1.0)
        nc.vector.memset(ones[:, :], 1.0)
        nc.vector.memset(sqNS[0:64, 0:128], 0.0)
        nc.vector.memset(sqNS[64:128, 8192:8320], 0.0)

        # acc bank: rows 0 of four 512-col regions hold the per-scale
        # energy accumulators; warmup/guard matmuls write the (otherwise
        # unused) full-partition region before any reduce starts
        accbank = psepool.tile([P, 4 * 512], F32, name="accbank")

        # warmup matmuls on DVE-initialized scratch: ramp the PE clock
        # while input DMA is still in flight (no DMA dependency at all)
        for _ in range(4):
            nc.tensor.matmul(
                accbank[:, 0:512], scr[:, 0:128], scr[:, :], start=True, stop=True,
                skip_group_check=True,
            )

        # guard matmuls: one per input chunk; each carries that chunk's DMA
        # sem wait so real matmuls below never need a second wait
        def guard(chunk_idx):
            # writes partitions 64.. only: partition 0 holds the live
            # energy accumulators, which guards must not reset
            lo = CHUNKS[chunk_idx][0]
            nc.tensor.matmul(
                accbank[64:128, 0:64], xgsb[:, lo : lo + 64], xgsb[:, lo : lo + 64],
                start=True, stop=True, skip_group_check=True,
            )

        # per-scale PSUM energy accumulator views [1, 512]
        accs = [accbank[0:1, 512 * i : 512 * i + 512] for i in range(4)]
        acc_started = [False] * 4
        acc_nred = [9, 2, 2, 2]      # reduces per acc
        acc_done = [0] * 4

        lhsT_NS = xgsb[:, W_NS : W_NS + 256].rearrange("p (j t) -> p j t", j=2)
        lhsT_S4 = [
            [
                xgsb[:, W_S4[k] + 256 * q : W_S4[k] + 256 * q + 256].rearrange(
                    "p (j t) -> p j t", j=2
                )
                for q in range(3)
            ]
            for k in range(3)
        ]
        ones_dr = bass.AP(
            ones.tensor, ones[:, :].offset, [list(ones[:, :].ap[0]), [16, 2], [1, 1]]
        )

        def square(eng_act, dst_sq, lo, n, pt, plo=0, phi=P):
            """square psum [plo:phi, 0:n] into dst_sq[plo:phi, lo:lo+n]."""
            if eng_act:
                nc.scalar.square(dst_sq[plo:phi, lo : lo + n], pt[plo:phi, 0:n])
            else:
                cp = cppool.tile([P, 512], BF16)
                nc.vector.tensor_copy(cp[plo:phi, 0:n], pt[plo:phi, 0:n])
                nc.vector.tensor_mul(
                    dst_sq[plo:phi, lo : lo + n], cp[plo:phi, 0:n], cp[plo:phi, 0:n]
                )

        def reduce(ai, rhs_ap, n, dr=True):
            acc_done[ai] += 1
            nc.tensor.matmul(
                accs[ai][:, 0:n], ones_dr if dr else ones[:, 0:1], rhs_ap,
                start=not acc_started[ai], stop=acc_done[ai] == acc_nred[ai],
                perf_mode=DR if dr else None, skip_group_check=True,
            )
            acc_started[ai] = True

        # ---- helpers for schedule ----
        def conv_NS(g):
            pt = pspool.tile([P, 512], F32, tag="conv")
            nc.tensor.matmul(
                pt[:, :], lhsT_NS, _xw4(xgsb, XB + 512 * g, P),
                start=True, stop=True, perf_mode=DR,
            )
            act = NS_PAIR_ACT[g // 2]
            if g == 0:
                # m=-1 edge: only out-times 0..63 (partitions 64:) valid
                square(act, sqNS, 0, 128, pt, plo=64)
                sl = sqNS[:, 128:512]
                if act:
                    nc.scalar.square(sl, pt[:, 128:512])
                else:
                    cp = cppool.tile([P, 512], BF16)
                    nc.vector.tensor_copy(cp[:, 0:384], pt[:, 128:512])
                    nc.vector.tensor_mul(sl, cp[:, 0:384], cp[:, 0:384])
            else:
                square(act, sqNS, 512 * g, 512, pt)

        def conv_S4(k, T):
            pt = pspool.tile([P, 512], F32, tag="conv")
            for q in range(3):
                nc.tensor.matmul(
                    pt[:, :], lhsT_S4[k][q],
                    _xw4(xgsb, XB + P * (16 * T + 2 * q), 4 * P),
                    start=q == 0, stop=q == 2, perf_mode=DR,
                )
            square(S4_PAIR_ACT[k][T // 2], sqS4[k], 512 * T, 512, pt)

        def conv_NS_tail():
            pt = pspool.tile([P, 512], F32, tag="conv")
            nc.tensor.matmul(
                pt[:, 0:128], lhsT_NS, _xw3(xgsb, XB + 512 * 16, 128),
                start=True, stop=True, perf_mode=DR,
            )
            square(NS_TAIL_ACT, sqNS, 8192, 128, pt, phi=64)

        def red_NS(r):
            reduce(0, _pair_ap(sqNS, 1024 * r, 512, 512), 512)

        def red_S4(k, pair):
            reduce(1 + k, _pair_ap(sqS4[k], 1024 * pair, 512, 512), 512)

        # ---- main schedule ----
        # quarter 0 (reduces for a quarter run ~2 convs into the next
        # quarter so they never stall on the squares they consume)
        guard(0)                      # small-scale lhsT
        guard(1)                      # x blocks 0..9
        conv_NS(0); conv_NS(1)
        guard(3)                      # x blocks 10..18
        conv_NS(2); conv_NS(3)
        guard(2)                      # strided lhsT
        conv_S4(0, 0); conv_S4(1, 0); conv_S4(2, 0)
        # quarter 1
        guard(4)                      # x blocks 19..34
        conv_NS(4); conv_NS(5)
        red_NS(0)
        conv_NS(6); conv_NS(7)
        red_NS(1)
        conv_S4(0, 1); conv_S4(1, 1); conv_S4(2, 1)
        # quarter 2
        guard(5)                      # x blocks 35..65
        conv_NS(8); conv_NS(9)
        red_NS(2); red_NS(3)
        conv_NS(10); conv_NS(11)
        red_S4(0, 0); red_S4(1, 0); red_S4(2, 0)
        conv_S4(0, 2); conv_S4(1, 2); conv_S4(2, 2)
        # quarter 3: small scale first so its accumulator finishes while
        # the strided convs still stream; copies overlap the tail
        conv_NS(12); conv_NS(13)
        red_NS(4)
        conv_NS(14); conv_NS(15)
        red_NS(5)
        conv_NS_tail()
        conv_S4(2, 3)
        red_NS(6)
        conv_S4(0, 3)
        red_NS(7)
        reduce(0, sqNS[:, 8192:8320], 128, dr=False)     # tail reduce
        nc.scalar.copy(rowout[:, 0:512], accs[0])        # acc0 out
        red_S4(2, 1)
        nc.vector.tensor_copy(rowout[:, 1536:2048], accs[3])   # acc3 out
        conv_S4(1, 3)
        red_S4(0, 1)
        nc.scalar.copy(rowout[:, 512:1024], accs[1])     # acc1 out
        red_S4(1, 1)
        nc.vector.tensor_copy(rowout[:, 1024:1536], accs[2])   # acc2 out
        nc.sync.dma_start(out=outp[:, 0:1024], in_=rowout[:, 0:1024])
        nc.scalar.dma_start(out=outp[:, 1024:2048], in_=rowout[:, 1024:2048])

    return nc


def _strip_pe_self_waits(nc: bass.Bass):
    """Drop PE-on-PE semaphore waits.  The PE executes its stream in
    order, so a WAW between two PE matmuls (psum buffer recycling) never
    needs a semaphore; the tile scheduler occasionally emits one anyway,
    which trips the walrus single-wait limit."""
    for blk in nc.m.functions[0].blocks:
        for ins in blk.instructions:
            si = getattr(ins, "sync_info", None)
            if si is None:
                continue
            waits = list(si.on_wait)
            if len(waits) <= 1:
                continue
            if ins.engine == mybir.EngineType.PE:
                keep = [w for w in waits if not w.ant_name.startswith("PE_")]
                if len(keep) < len(waits) and len(keep) <= 1:
                    si.on_wait = keep
    for blk in nc.m.functions[0].blocks:
        for ins in blk.instructions:
            si = getattr(ins, "sync_info", None)
            if si is not None and len(list(si.on_wait)) > 1:
                raise RuntimeError(f"multi-wait survives: {ins.name}")


_NC_CACHE = None


def _get_nc() -> bass.Bass:
    global _NC_CACHE
    if _NC_CACHE is None:
        _NC_CACHE = _build_nc()
        _strip_pe_self_waits(_NC_CACHE)
    return _NC_CACHE


def kernel(x: np.ndarray, scale_weights: np.ndarray, _trace: bool = False) -> np.ndarray:
    global LAST_RESULTS
    import ml_dtypes

    e4 = ml_dtypes.float8_e4m3fn
    x = np.asarray(x, dtype=np.float32)
    scale_weights = np.asarray(scale_weights, dtype=np.float32)
    assert x.shape == (P, NT) and scale_weights.shape == (S_TOTAL,)

    bank = _morlet_kernel_bank(S_TOTAL, NT)           # [32, 256] fp32
    gq = bank[:, ::-1].astype(e4).astype(np.float32)  # quantized g' rows

    xq8 = x.T.astype(e4)                              # [NT, P] fp8
    # x layout: xcol[p, 128*I + b] = xpad[128*I + p, b]
    xpad = np.zeros((NBLK * P, P), dtype=e4)
    xpad[P : P + NT, :] = xq8
    xcol = xpad.reshape(NBLK, P, P).transpose(1, 0, 2).reshape(P, NBLK * P)

    xgs = []
    for c in range(N_CORES):
        sc = _core_scales(c)
        buf = np.empty((P, NCOL), dtype=e4)
        buf[:, W_NS : W_NS + 256] = _lhsT_small(gq[sc[0]]).astype(e4)
        for k in range(3):
            for q in range(3):
                buf[:, W_S4[k] + 256 * q : W_S4[k] + 256 * q + 256] = _lhsT_s4(
                    gq[sc[1 + k]], q
                ).astype(e4)
        buf[:, XB:] = xcol
        xgs.append(buf)

    nc = _get_nc()
    in_maps = [{"xg": xgs[c]} for c in range(N_CORES)]
    res = run_bass_kernel_spmd(nc, in_maps, list(range(N_CORES)), trace=_trace)
    LAST_RESULTS = res

    # gather: core c rows = [small scale c, 8+3c, 9+3c, 10+3c]
    energy = np.zeros((P, S_TOTAL), dtype=np.float64)
    for c in range(N_CORES):
        vals = res.results[c]["outp"].reshape(4, 4, P).astype(np.float64).sum(axis=1)
        sc = _core_scales(c)
        energy[:, sc[0]] = vals[0] / NT
        for k in range(3):
            energy[:, sc[1 + k]] = vals[1 + k] * 4.0 / NT

    # exact correction of the deterministic fp8 norm bias
    w2 = (bank.astype(np.float64) ** 2).sum(1)
    wq2 = (gq.astype(np.float64) ** 2).sum(1)
    mx2 = (x.astype(np.float64) ** 2).mean(1)
    mxq2 = (xq8.T.astype(np.float64) ** 2).mean(1)
    energy = energy * (mx2[:, None] * w2[None, :]) / (mxq2[:, None] * wq2[None, :])

    w = scale_weights.astype(np.float64)
    e = np.exp(w - w.max())
    sm = e / e.sum()
    return (energy * sm[None, :]).astype(np.float32)


if __name__ == "__main__":
    rng = np.random.default_rng(0)
    x = rng.standard_normal((P, NT), dtype=np.float32)
    sw = rng.standard_normal(S_TOTAL, dtype=np.float32)
    out = kernel(x, sw)
    print("kernel output shape:", out.shape, out.dtype)
